# revision 5
# baseline (speedup 1.0000x reference)
"""DirGCNConv Trainium2 kernel (8 NeuronCores, data parallel).

Math:  out = a*(A @ x) @ W_ds + (1-a)*(A^T @ x) @ W_sd + a*b_ds + (1-a)*b_sd
where A[r,c] = sum_{e=(r,c)} w_e,  w_e = rsqrt(out_deg[r]) * rsqrt(in_deg[c]).

Device strategy per core (core m owns output nodes [m*12544, (m+1)*12544)):
  - Host sorts each direction's edges by destination, groups them into
    128-destination windows, and within each window buckets sources into
    32768-row ranges (so indices fit int16 for dma_gather).
  - Per (window, bucket) section sizes are padded to the max over the 8
    cores, giving an identical program structure on every core (SPMD);
    per-core differences live only in the index / weight metadata.
  - Gathers run as 1024-index dma_gather calls round-robined over 4 SWDGE
    queues (the Q7 descriptor-generation path is the bottleneck; 4 queues
    give ~205 GB/s effective).
  - Scatter-add = PE matmuls: for each 128-edge block, a one-hot matrix
    built on the vector engine (iota == dst_local) * w routes edge rows
    into the window's PSUM accumulator [128 feat, WDST dst].
  - Per window: two small matmuls apply alpha*W_ds / (1-alpha)*W_sd and the
    result [WDST, 128] is written out.
"""

import sys

for _p in ("/opt/trn_rl_repo",):
    if _p not in sys.path:
        sys.path.insert(0, _p)

from contextlib import ExitStack

import numpy as np

import concourse.bass as bass
import concourse.tile as tile
from concourse import bacc, mybir

F = 128
P = 128
N = 100000
NCORES = 8
NPC = 12544
ALPHA = 0.5

WDST = 128            # destinations per window (one-hot width, <= 512/4 fp32 psum)
BUCKET = 32768        # source rows per gather sub-table (int16 index range)
CALL_IDX = 1024       # indices per dma_gather call (single-packet limit)
NQ = 4                # SWDGE queues (ucode max)
DTYPE_STR = "bf16"    # "bf16" or "f32" for gather/one-hot/matmul phase 1

f32 = mybir.dt.float32
i16 = mybir.dt.int16


def _dt():
    return mybir.dt.bfloat16 if DTYPE_STR == "bf16" else mybir.dt.float32


def _np_dt():
    if DTYPE_STR == "bf16":
        import ml_dtypes

        return ml_dtypes.bfloat16
    return np.float32


def _wrap_idxs_call(vals):
    """One call's indices: index j -> partition j%16, col j//16; 8 replicas.
    Always occupies CALL_IDX//16 columns (trailing -1 fill is never read)."""
    n = len(vals)
    cols = CALL_IDX // 16
    arr = np.full(cols * 16, -1, np.int16)
    arr[:n] = vals
    block = arr.reshape(cols, 16).T
    return np.tile(block, (8, 1))  # [128, cols]


class Plan:
    """Core-independent program structure for one direction."""

    def __init__(self, caps, wdst, nwin):
        self.caps = caps                      # [nwin, nb] section capacities
        nb = caps.shape[1]
        self.nb = nb
        self.offs = []                        # per bucket: section offsets [nwin+1]
        self.lpad = []                        # per bucket: padded stream length
        self.ncalls = []
        for b in range(nb):
            o = np.concatenate([[0], np.cumsum(caps[:, b])]).astype(np.int64)
            self.offs.append(o)
            lp = int(-(-o[-1] // 128) * 128)
            self.lpad.append(lp)
            self.ncalls.append(-(-lp // CALL_IDX))
        # jobs: (window, bucket, block) incidences in window-major emission order
        self.jobs = []
        self.win_jobs = [[] for _ in range(nwin)]
        for w in range(nwin):
            for b in range(nb):
                cap = int(caps[w, b])
                if cap == 0:
                    continue
                lo, hi = int(self.offs[b][w]), int(self.offs[b][w]) + cap
                for blk in range(lo // 128, (hi - 1) // 128 + 1):
                    self.win_jobs[w].append((b, blk, len(self.jobs)))
                    self.jobs.append((w, b, blk))
        self.njobs = len(self.jobs)


def _prepare(x, edge_index, ncores, npc, wdst, bucket):
    """Host prep. Returns (plans {dir: Plan}, in_maps list, nwin)."""
    n = x.shape[0]
    nwin = npc // wdst
    nb = -(-n // bucket)
    row = edge_index[0].astype(np.int64)
    col = edge_index[1].astype(np.int64)
    deg_r = np.bincount(row, minlength=n).astype(np.float32)
    deg_c = np.bincount(col, minlength=n).astype(np.float32)
    ar = np.zeros(n, np.float32)
    m = deg_r > 0
    ar[m] = 1.0 / np.sqrt(deg_r[m])
    ac = np.zeros(n, np.float32)
    m = deg_c > 0
    ac[m] = 1.0 / np.sqrt(deg_c[m])
    wgt = (ar[row] * ac[col]).astype(np.float32)

    np_dt = _np_dt()
    dirs = {}
    percore = {}
    for key, (dst_all, src_all) in (("in", (row, col)), ("out", (col, row))):
        order = np.argsort(dst_all, kind="stable")
        d, s, ww = dst_all[order], src_all[order], wgt[order]
        counts = np.zeros((ncores, nwin, nb), np.int64)
        coredata = []
        for mc in range(ncores):
            lo = np.searchsorted(d, mc * npc)
            hi = np.searchsorted(d, (mc + 1) * npc)
            dl = d[lo:hi] - mc * npc
            sl = s[lo:hi]
            wl = ww[lo:hi]
            key2 = (dl // wdst) * nb + sl // bucket
            o2 = np.argsort(key2, kind="stable")
            dl, sl, wl, key2 = dl[o2], sl[o2], wl[o2], key2[o2]
            counts[mc] = np.bincount(key2, minlength=nwin * nb).reshape(nwin, nb)
            coredata.append((dl, sl, wl))
        caps = counts.max(axis=0)
        plan = Plan(caps, wdst, nwin)
        dirs[key] = plan
        percore[key] = (counts, coredata)

    in_maps = []
    for mc in range(ncores):
        im = {"x": np.ascontiguousarray(np.asarray(x, np.float32).astype(np_dt))}
        for key, plan in dirs.items():
            counts, coredata = percore[key]
            cnt = counts[mc]
            dl, sl, wl = coredata[mc]
            # per-core section start offsets in the *edge* array (sorted order)
            ecum = np.concatenate([[0], np.cumsum(cnt.reshape(-1))]).reshape(-1)
            # build padded streams per bucket
            idx_cols = []
            dst_stream = []
            w_stream = []
            for b in range(plan.nb):
                lp = plan.lpad[b]
                sv = np.zeros(lp, np.int64)
                dv = np.zeros(lp, np.float64)
                wv = np.zeros(lp, np.float64)
                for w in range(plan.caps.shape[0]):
                    c = int(cnt[w, b])
                    if c == 0:
                        continue
                    e0 = int(ecum[w * plan.nb + b])
                    o = int(plan.offs[b][w])
                    sv[o : o + c] = sl[e0 : e0 + c] - b * bucket
                    dv[o : o + c] = dl[e0 : e0 + c] - w * wdst
                    wv[o : o + c] = wl[e0 : e0 + c]
                dst_stream.append(dv)
                w_stream.append(wv)
                # wrap indices per call
                for ci in range(plan.ncalls[b]):
                    chunk = sv[ci * CALL_IDX : (ci + 1) * CALL_IDX]
                    idx_cols.append(_wrap_idxs_call(chunk.astype(np.int16)))
            im[f"idx_{key}"] = np.ascontiguousarray(np.concatenate(idx_cols, axis=1))
            # job metadata columns
            djob = np.zeros((128, plan.njobs), np.float64)
            wjob = np.zeros((128, plan.njobs), np.float64)
            for j, (w, b, blk) in enumerate(plan.jobs):
                lo = int(plan.offs[b][w])
                hi = lo + int(cnt[w, b])  # real edges only; cap padding stays 0
                a0, a1 = blk * 128, blk * 128 + 128
                v0, v1 = max(lo, a0), min(hi, a1)
                if v0 < v1:
                    djob[v0 - a0 : v1 - a0, j] = dst_stream[b][v0:v1]
                    wjob[v0 - a0 : v1 - a0, j] = w_stream[b][v0:v1]
            im[f"dst_{key}"] = np.ascontiguousarray(djob.astype(np.float32))
            im[f"wgt_{key}"] = np.ascontiguousarray(wjob.astype(np.float32))
        in_maps.append(im)
    return dirs, in_maps, nwin


def _build(plans, nwin, n_rows_x, npc, wdst):
    dt = _dt()
    nc = bacc.Bacc(
        "TRN2", target_bir_lowering=False, debug=False, num_swdge_queues=NQ
    )
    x_t = nc.dram_tensor("x", [n_rows_x, F], dt, kind="ExternalInput")
    meta_t = {}
    for key, plan in plans.items():
        icols = sum(plan.ncalls[b] * (CALL_IDX // 16) for b in range(plan.nb))
        meta_t[f"idx_{key}"] = nc.dram_tensor(
            f"idx_{key}", [P, icols], i16, kind="ExternalInput"
        )
        meta_t[f"dst_{key}"] = nc.dram_tensor(
            f"dst_{key}", [P, plan.njobs], f32, kind="ExternalInput"
        )
        meta_t[f"wgt_{key}"] = nc.dram_tensor(
            f"wgt_{key}", [P, plan.njobs], f32, kind="ExternalInput"
        )
    wds_t = nc.dram_tensor("wds", [F, F], f32, kind="ExternalInput")
    wsd_t = nc.dram_tensor("wsd", [F, F], f32, kind="ExternalInput")
    iota_t = nc.dram_tensor("iota", [P, wdst], f32, kind="ExternalInput")
    y_t = nc.dram_tensor("y", [npc, F], f32, kind="ExternalOutput")

    with tile.TileContext(nc) as tc, ExitStack() as ctx:
        const = ctx.enter_context(tc.tile_pool(name="const", bufs=1))
        gp = ctx.enter_context(tc.tile_pool(name="gb", bufs=3))
        ohp = ctx.enter_context(tc.tile_pool(name="oh", bufs=8))
        ysp = ctx.enter_context(tc.tile_pool(name="ys", bufs=4))
        osp = ctx.enter_context(tc.tile_pool(name="os", bufs=4))
        pp = ctx.enter_context(tc.tile_pool(name="ps", bufs=2, space="PSUM"))

        iota_sb = const.tile([P, wdst], f32)
        nc.sync.dma_start(iota_sb[:], iota_t[:])
        wds_sb = const.tile([F, F], f32)
        nc.sync.dma_start(wds_sb[:], wds_t[:])
        wsd_sb = const.tile([F, F], f32)
        nc.sync.dma_start(wsd_sb[:], wsd_t[:])
        meta_sb = {}
        for k, t in meta_t.items():
            mt = const.tile(list(t.shape), t.dtype, tag=k)
            nc.sync.dma_start(mt[:], t[:])
            meta_sb[k] = mt

        # per (dir, bucket): column offset of each call in idx tensor, call tiles
        call_state = {}
        for key, plan in plans.items():
            coff = 0
            st = []
            for b in range(plan.nb):
                st.append({"coff": coff, "emitted": 0, "tiles": {}})
                coff += plan.ncalls[b] * (CALL_IDX // 16)
            call_state[key] = st

        qctr = [0]

        def emit_call(key, plan, b, ci):
            st = call_state[key][b]
            lp = plan.lpad[b]
            nidx = min(CALL_IDX, lp - ci * CALL_IDX)
            nblk = -(-nidx // 128)
            gt = gp.tile([P, 8 * F], dt, tag=f"g_{key}_{b}")
            ccols = CALL_IDX // 16
            rows0 = b * BUCKET
            rows1 = min(n_rows_x, (b + 1) * BUCKET)
            nc.gpsimd.dma_gather(
                out_ap=gt[:, : nblk * F].rearrange("p (k e) -> p k e", e=F),
                in_ap=x_t[rows0:rows1, :],
                idxs_ap=meta_sb[f"idx_{key}"][
                    :, st["coff"] + ci * ccols : st["coff"] + ci * ccols + (-(-nidx // 16))
                ],
                num_idxs=nidx,
                num_idxs_reg=nidx,
                elem_size=F,
                queue_num=qctr[0] % NQ,
            )
            qctr[0] += 1
            st["tiles"][ci] = gt
            st["emitted"] = ci + 1

        for w in range(nwin):
            yps = {}
            for key, plan in plans.items():
                st = call_state[key]
                wj = plan.win_jobs[w]
                ps = pp.tile([P, wdst], f32, tag=f"y{key}")
                for i, (b, blk, j) in enumerate(wj):
                    ci = blk // 8
                    while st[b]["emitted"] <= ci:
                        emit_call(key, plan, b, st[b]["emitted"])
                    gt = st[b]["tiles"][ci]
                    lb = blk % 8
                    oh = ohp.tile([P, wdst], dt, tag="oh")
                    nc.vector.tensor_scalar(
                        out=oh[:],
                        in0=iota_sb[:],
                        scalar1=meta_sb[f"dst_{key}"][:, j : j + 1],
                        scalar2=meta_sb[f"wgt_{key}"][:, j : j + 1],
                        op0=mybir.AluOpType.is_equal,
                        op1=mybir.AluOpType.mult,
                    )
                    nc.tensor.matmul(
                        ps[:],
                        lhsT=gt[:, lb * F : (lb + 1) * F],
                        rhs=oh[:],
                        start=(i == 0),
                        stop=(i == len(wj) - 1),
                    )
                yps[key] = ps
            ysb = {}
            for key in plans:
                sb = ysp.tile([P, wdst], f32, tag=f"ysb{key}")
                if plans[key].win_jobs[w]:
                    nc.scalar.copy(sb[:], yps[key][:])
                else:
                    nc.vector.memset(sb[:], 0.0)
                ysb[key] = sb
            ops_ = pp.tile([wdst, F], f32, tag="o")
            nc.tensor.matmul(
                ops_[:], lhsT=ysb["in"][:], rhs=wds_sb[:], start=True, stop=False
            )
            nc.tensor.matmul(
                ops_[:], lhsT=ysb["out"][:], rhs=wsd_sb[:], start=False, stop=True
            )
            ot = osp.tile([wdst, F], f32, tag="ot")
            nc.scalar.copy(ot[:], ops_[:])
            nc.sync.dma_start(y_t[w * wdst : (w + 1) * wdst, :], ot[:])
    nc.compile()
    return nc


def _make_in_maps(x, edge_index, W_sd, W_ds, ncores, npc, wdst, bucket):
    plans, in_maps, nwin = _prepare(
        np.asarray(x), np.asarray(edge_index), ncores, npc, wdst, bucket
    )
    wds = np.ascontiguousarray(ALPHA * np.asarray(W_ds), dtype=np.float32)
    wsd = np.ascontiguousarray((1.0 - ALPHA) * np.asarray(W_sd), dtype=np.float32)
    iota = np.tile(np.arange(wdst, dtype=np.float32), (P, 1))
    for im in in_maps:
        im["wds"] = wds
        im["wsd"] = wsd
        im["iota"] = np.ascontiguousarray(iota)
    return plans, in_maps, nwin


def kernel(x, edge_index, W_sd, b_sd, W_ds, b_ds):
    from concourse.bass_utils import run_bass_kernel_spmd

    x = np.asarray(x, dtype=np.float32)
    edge_index = np.asarray(edge_index)
    plans, in_maps, nwin = _make_in_maps(
        x, edge_index, W_sd, W_ds, NCORES, NPC, WDST, BUCKET
    )
    nc = _build(plans, nwin, x.shape[0], NPC, WDST)
    res = run_bass_kernel_spmd(nc, in_maps, list(range(NCORES)))
    y = np.concatenate(
        [np.asarray(res.results[m]["y"]) for m in range(NCORES)], axis=0
    )[:N]
    bias = ALPHA * np.asarray(b_ds) + (1.0 - ALPHA) * np.asarray(b_sd)
    return (y + bias[None, :]).astype(np.float32)


# revision 6
# speedup vs baseline: 1.2653x; 1.2653x over previous
"""DirGCNConv Trainium2 kernel (8 NeuronCores, data parallel).

Math:  out = a*(A @ x) @ W_ds + (1-a)*(A^T @ x) @ W_sd + a*b_ds + (1-a)*b_sd
where A[r,c] = sum_{e=(r,c)} w_e,  w_e = ar[r] * ac[c],
      ar = rsqrt(out_deg), ac = rsqrt(in_deg).

w factorizes per edge: the source factor is folded into the gather table on
the host (x_in = ac*x for the IN direction, x_out = ar*x for OUT) and the
destination factor is applied per-partition when evacuating the per-window
result.  The on-device scatter-add therefore uses PURE 0/1 one-hot matrices.

Device strategy per core (core m owns output nodes [m*12544, (m+1)*12544)):
  - Host sorts each direction's edges by destination, groups them into
    WDST-destination windows, and within each window buckets sources into
    32768-row ranges (indices fit int16 for dma_gather).
  - Per (window, bucket) section sizes are padded to the max over the 8
    cores -> identical SPMD program structure; per-core differences live
    only in index/metadata values.
  - Gathers: 1024-index dma_gather calls round-robined over 4 SWDGE queues
    (Q7 descriptor generation is the hard floor: ~2.2-2.6ns/row at 4 queues).
  - Scatter-add: per 128-edge block, a one-hot (iota == dst_local) built on
    the vector engine routes edge rows into the window PSUM [feat, WDST]
    via one PE matmul (fp16 operands, fp32 accumulate).
  - Per window: yin/yout -> two fp32 matmuls with alpha*W_ds / (1-a)*W_sd,
    per-destination degree scales applied during the ACT evacuation, DVE
    add, DMA out.
"""

import sys

for _p in ("/opt/trn_rl_repo",):
    if _p not in sys.path:
        sys.path.insert(0, _p)

from contextlib import ExitStack

import numpy as np

import concourse.bass as bass
import concourse.tile as tile
from concourse import bacc, mybir

F = 128
P = 128
N = 100000
NCORES = 8
NPC = 12544
ALPHA = 0.5

WDST = 128            # destinations per window (one-hot width)
BUCKET = 32768        # source rows per gather sub-table (int16 index range)
CALL_IDX = 1024       # indices per dma_gather call (single-packet limit)
NQ = 4                # SWDGE queues (ucode max)
DTYPE_STR = "f16"     # "f16" | "bf16" | "f32": gather table / one-hot / phase-1 matmul

f32 = mybir.dt.float32
i16 = mybir.dt.int16


def _dt():
    return {
        "f16": mybir.dt.float16,
        "bf16": mybir.dt.bfloat16,
        "f32": mybir.dt.float32,
    }[DTYPE_STR]


def _np_dt():
    if DTYPE_STR == "f16":
        return np.float16
    if DTYPE_STR == "bf16":
        import ml_dtypes

        return ml_dtypes.bfloat16
    return np.float32


def _wrap_idxs_call(vals):
    """One call's indices: index j -> partition j%16, col j//16; 8 replicas.
    Always occupies CALL_IDX//16 columns (trailing -1 fill is never read)."""
    n = len(vals)
    cols = CALL_IDX // 16
    arr = np.full(cols * 16, -1, np.int16)
    arr[:n] = vals
    block = arr.reshape(cols, 16).T
    return np.tile(block, (8, 1))  # [128, cols]


class Plan:
    """Core-independent program structure for one direction."""

    def __init__(self, caps, wdst, nwin):
        self.caps = caps                      # [nwin, nb] section capacities
        nb = caps.shape[1]
        self.nb = nb
        self.offs = []
        self.lpad = []
        self.ncalls = []
        for b in range(nb):
            o = np.concatenate([[0], np.cumsum(caps[:, b])]).astype(np.int64)
            self.offs.append(o)
            lp = int(-(-o[-1] // 128) * 128)
            self.lpad.append(lp)
            self.ncalls.append(-(-lp // CALL_IDX))
        # jobs: (window, bucket, block) incidences in window-major order
        self.jobs = []
        self.win_jobs = [[] for _ in range(nwin)]
        for w in range(nwin):
            for b in range(nb):
                cap = int(caps[w, b])
                if cap == 0:
                    continue
                lo, hi = int(self.offs[b][w]), int(self.offs[b][w]) + cap
                for blk in range(lo // 128, (hi - 1) // 128 + 1):
                    self.win_jobs[w].append((b, blk, len(self.jobs)))
                    self.jobs.append((w, b, blk))
        self.njobs = len(self.jobs)


def _prepare(x, edge_index, ncores, npc, wdst, bucket):
    """Host prep. Returns (plans {dir: Plan}, in_maps list, nwin)."""
    n = x.shape[0]
    nwin = npc // wdst
    nb = -(-n // bucket)
    row = edge_index[0].astype(np.int64)
    col = edge_index[1].astype(np.int64)
    deg_r = np.bincount(row, minlength=n).astype(np.float32)
    deg_c = np.bincount(col, minlength=n).astype(np.float32)
    ar = np.zeros(n, np.float32)
    m = deg_r > 0
    ar[m] = 1.0 / np.sqrt(deg_r[m])
    ac = np.zeros(n, np.float32)
    m = deg_c > 0
    ac[m] = 1.0 / np.sqrt(deg_c[m])

    np_dt = _np_dt()
    x = np.asarray(x, np.float32)
    tables = {
        "in": np.ascontiguousarray((x * ac[:, None]).astype(np_dt)),
        "out": np.ascontiguousarray((x * ar[:, None]).astype(np_dt)),
    }
    # per-destination evacuation scales, padded to ncores*npc nodes
    npad = ncores * npc
    a_in = np.zeros(npad, np.float32)
    a_in[:n] = ar           # IN: dst = row -> ar[dst]
    a_out = np.zeros(npad, np.float32)
    a_out[:n] = ac          # OUT: dst = col -> ac[dst]

    dirs = {}
    percore = {}
    for key, (dst_all, src_all) in (("in", (row, col)), ("out", (col, row))):
        order = np.argsort(dst_all, kind="stable")
        d, s = dst_all[order], src_all[order]
        counts = np.zeros((ncores, nwin, nb), np.int64)
        coredata = []
        for mc in range(ncores):
            lo = np.searchsorted(d, mc * npc)
            hi = np.searchsorted(d, (mc + 1) * npc)
            dl = d[lo:hi] - mc * npc
            sl = s[lo:hi]
            key2 = (dl // wdst) * nb + sl // bucket
            o2 = np.argsort(key2, kind="stable")
            dl, sl, key2 = dl[o2], sl[o2], key2[o2]
            counts[mc] = np.bincount(key2, minlength=nwin * nb).reshape(nwin, nb)
            coredata.append((dl, sl))
        caps = counts.max(axis=0)
        plan = Plan(caps, wdst, nwin)
        dirs[key] = plan
        percore[key] = (counts, coredata)

    in_maps = []
    for mc in range(ncores):
        im = {"x_in": tables["in"], "x_out": tables["out"]}
        for key, plan in dirs.items():
            counts, coredata = percore[key]
            cnt = counts[mc]
            dl, sl = coredata[mc]
            ecum = np.concatenate([[0], np.cumsum(cnt.reshape(-1))])
            idx_cols = []
            dst_stream = []
            for b in range(plan.nb):
                lp = plan.lpad[b]
                sv = np.zeros(lp, np.int64)
                dv = np.zeros(lp, np.float64)
                for w in range(plan.caps.shape[0]):
                    c = int(cnt[w, b])
                    if c == 0:
                        continue
                    e0 = int(ecum[w * plan.nb + b])
                    o = int(plan.offs[b][w])
                    sv[o : o + c] = sl[e0 : e0 + c] - b * bucket
                    dv[o : o + c] = dl[e0 : e0 + c] - w * wdst
                dst_stream.append(dv)
                for ci in range(plan.ncalls[b]):
                    chunk = sv[ci * CALL_IDX : (ci + 1) * CALL_IDX]
                    idx_cols.append(_wrap_idxs_call(chunk.astype(np.int16)))
            im[f"idx_{key}"] = np.ascontiguousarray(np.concatenate(idx_cols, axis=1))
            # job dst columns; -1 (never matches iota) for slots outside the
            # job's window or beyond the real edge count
            djob = np.full((128, plan.njobs), -1.0, np.float64)
            for j, (w, b, blk) in enumerate(plan.jobs):
                lo = int(plan.offs[b][w])
                hi = lo + int(cnt[w, b])
                a0 = blk * 128
                v0, v1 = max(lo, a0), min(hi, a0 + 128)
                if v0 < v1:
                    djob[v0 - a0 : v1 - a0, j] = dst_stream[b][v0:v1]
            im[f"dst_{key}"] = np.ascontiguousarray(djob.astype(np.float32))
        # evacuation scale columns: [wdst, nwin] per dir
        base = mc * npc
        im["asc_in"] = np.ascontiguousarray(
            a_in[base : base + npc].reshape(nwin, wdst).T.astype(np.float32)
        )
        im["asc_out"] = np.ascontiguousarray(
            a_out[base : base + npc].reshape(nwin, wdst).T.astype(np.float32)
        )
        in_maps.append(im)
    return dirs, in_maps, nwin


def _build(plans, nwin, n_rows_x, npc, wdst):
    dt = _dt()
    nc = bacc.Bacc(
        "TRN2", target_bir_lowering=False, debug=False, num_swdge_queues=NQ
    )
    x_t = {
        "in": nc.dram_tensor("x_in", [n_rows_x, F], dt, kind="ExternalInput"),
        "out": nc.dram_tensor("x_out", [n_rows_x, F], dt, kind="ExternalInput"),
    }
    meta_t = {}
    for key, plan in plans.items():
        icols = sum(plan.ncalls[b] * (CALL_IDX // 16) for b in range(plan.nb))
        meta_t[f"idx_{key}"] = nc.dram_tensor(
            f"idx_{key}", [P, icols], i16, kind="ExternalInput"
        )
        meta_t[f"dst_{key}"] = nc.dram_tensor(
            f"dst_{key}", [P, plan.njobs], f32, kind="ExternalInput"
        )
        meta_t[f"asc_{key}"] = nc.dram_tensor(
            f"asc_{key}", [wdst, nwin], f32, kind="ExternalInput"
        )
    wds_t = nc.dram_tensor("wds", [F, F], f32, kind="ExternalInput")
    wsd_t = nc.dram_tensor("wsd", [F, F], f32, kind="ExternalInput")
    iota_t = nc.dram_tensor("iota", [P, wdst], dt, kind="ExternalInput")
    y_t = nc.dram_tensor("y", [npc, F], f32, kind="ExternalOutput")

    with tile.TileContext(nc) as tc, ExitStack() as ctx:
        const = ctx.enter_context(tc.tile_pool(name="const", bufs=1))
        gp = ctx.enter_context(tc.tile_pool(name="gb", bufs=3))
        ohp = ctx.enter_context(tc.tile_pool(name="oh", bufs=8))
        ysp = ctx.enter_context(tc.tile_pool(name="ys", bufs=4))
        osp = ctx.enter_context(tc.tile_pool(name="os", bufs=4))
        pp = ctx.enter_context(tc.tile_pool(name="ps", bufs=2, space="PSUM"))

        iota_sb = const.tile([P, wdst], dt)
        nc.sync.dma_start(iota_sb[:], iota_t[:])
        wds_sb = const.tile([F, F], f32)
        nc.sync.dma_start(wds_sb[:], wds_t[:])
        wsd_sb = const.tile([F, F], f32)
        nc.sync.dma_start(wsd_sb[:], wsd_t[:])
        meta_sb = {}
        for k, t in meta_t.items():
            mt = const.tile(list(t.shape), t.dtype, tag=k)
            nc.sync.dma_start(mt[:], t[:])
            meta_sb[k] = mt

        call_state = {}
        for key, plan in plans.items():
            coff = 0
            st = []
            for b in range(plan.nb):
                st.append({"coff": coff, "emitted": 0, "tiles": {}})
                coff += plan.ncalls[b] * (CALL_IDX // 16)
            call_state[key] = st

        qctr = [0]

        def emit_call(key, plan, b, ci):
            st = call_state[key][b]
            lp = plan.lpad[b]
            nidx = min(CALL_IDX, lp - ci * CALL_IDX)
            nblk = -(-nidx // 128)
            gt = gp.tile([P, 8 * F], dt, tag=f"g_{key}_{b}")
            ccols = CALL_IDX // 16
            rows0 = b * BUCKET
            rows1 = min(n_rows_x, (b + 1) * BUCKET)
            nc.gpsimd.dma_gather(
                out_ap=gt[:, : nblk * F].rearrange("p (k e) -> p k e", e=F),
                in_ap=x_t[key][rows0:rows1, :],
                idxs_ap=meta_sb[f"idx_{key}"][
                    :, st["coff"] + ci * ccols : st["coff"] + ci * ccols + nidx // 16
                ],
                num_idxs=nidx,
                num_idxs_reg=nidx,
                elem_size=F,
                queue_num=qctr[0] % NQ,
            )
            qctr[0] += 1
            st["tiles"][ci] = gt
            st["emitted"] = ci + 1

        for w in range(nwin):
            yps = {}
            for key, plan in plans.items():
                st = call_state[key]
                wj = plan.win_jobs[w]
                ps = pp.tile([P, wdst], f32, tag=f"y{key}")
                for i, (b, blk, j) in enumerate(wj):
                    ci = blk // 8
                    while st[b]["emitted"] <= ci:
                        emit_call(key, plan, b, st[b]["emitted"])
                    gt = st[b]["tiles"][ci]
                    lb = blk % 8
                    oh = ohp.tile([P, wdst], dt, tag="oh")
                    nc.vector.tensor_scalar(
                        out=oh[:],
                        in0=iota_sb[:],
                        scalar1=meta_sb[f"dst_{key}"][:, j : j + 1],
                        scalar2=None,
                        op0=mybir.AluOpType.is_equal,
                    )
                    nc.tensor.matmul(
                        ps[:],
                        lhsT=gt[:, lb * F : (lb + 1) * F],
                        rhs=oh[:],
                        start=(i == 0),
                        stop=(i == len(wj) - 1),
                    )
                yps[key] = ps
            ysb = {}
            for key in plans:
                sb = ysp.tile([P, wdst], f32, tag=f"ysb{key}")
                nc.scalar.copy(sb[:], yps[key][:])
                ysb[key] = sb
            o1 = pp.tile([wdst, F], f32, tag="o1")
            nc.tensor.matmul(o1[:], lhsT=ysb["in"][:], rhs=wds_sb[:],
                             start=True, stop=True)
            o2 = pp.tile([wdst, F], f32, tag="o2")
            nc.tensor.matmul(o2[:], lhsT=ysb["out"][:], rhs=wsd_sb[:],
                             start=True, stop=True)
            s1 = osp.tile([wdst, F], f32, tag="s1")
            nc.scalar.mul(s1[:], o1[:], meta_sb["asc_in"][:, w : w + 1])
            s2 = osp.tile([wdst, F], f32, tag="s2")
            nc.scalar.mul(s2[:], o2[:], meta_sb["asc_out"][:, w : w + 1])
            ot = osp.tile([wdst, F], f32, tag="ot")
            nc.vector.tensor_add(ot[:], s1[:], s2[:])
            nc.sync.dma_start(y_t[w * wdst : (w + 1) * wdst, :], ot[:])
    nc.compile()
    return nc


def _make_in_maps(x, edge_index, W_sd, W_ds, ncores, npc, wdst, bucket):
    plans, in_maps, nwin = _prepare(
        np.asarray(x), np.asarray(edge_index), ncores, npc, wdst, bucket
    )
    wds = np.ascontiguousarray(ALPHA * np.asarray(W_ds), dtype=np.float32)
    wsd = np.ascontiguousarray((1.0 - ALPHA) * np.asarray(W_sd), dtype=np.float32)
    iota = np.ascontiguousarray(
        np.tile(np.arange(wdst, dtype=np.float32), (P, 1)).astype(_np_dt())
    )
    for im in in_maps:
        im["wds"] = wds
        im["wsd"] = wsd
        im["iota"] = iota
    return plans, in_maps, nwin


def kernel(x, edge_index, W_sd, b_sd, W_ds, b_ds):
    from concourse.bass_utils import run_bass_kernel_spmd

    x = np.asarray(x, dtype=np.float32)
    edge_index = np.asarray(edge_index)
    plans, in_maps, nwin = _make_in_maps(
        x, edge_index, W_sd, W_ds, NCORES, NPC, WDST, BUCKET
    )
    nc = _build(plans, nwin, x.shape[0], NPC, WDST)
    res = run_bass_kernel_spmd(nc, in_maps, list(range(NCORES)))
    y = np.concatenate(
        [np.asarray(res.results[m]["y"]) for m in range(NCORES)], axis=0
    )[:N]
    bias = ALPHA * np.asarray(b_ds) + (1.0 - ALPHA) * np.asarray(b_sd)
    return (y + bias[None, :]).astype(np.float32)
